# revision 21
# baseline (speedup 1.0000x reference)
"""Trainium2 Bass kernel for nn_DGN6 (gnn_message_passing).

Reference computation (per batch element, 3 rounds with K = 4, 8, 16):
    S = h @ h.T; causal top-K neighbors per row; msg = masked mean of
    neighbor rows; h = mom*h + (1-mom)*gelu((mix*h + (1-mix)*msg)*gain + bias)
Output: (h - x) * scale.

Distribution: data-parallel over B (2 batches); within a batch, PARTIAL
REPLICATION over its 4 cores.  The leading 8 row-blocks (of 16 x 128
rows) are updated identically on every core of the group ("repl" blocks;
their inputs are causally local, so they need NO communication).  Blocks
8-15 are sharded as two tail slots per core (core cc owns blocks 8+cc
and 12+cc) and exchanged between rounds with two pipelined f32
AllGathers (2MB out each), which mostly hide behind the replicated
compute.  This trades redundant FLOPs for a large cut in collective
cost, which dominates the roofline here.

Top-k SELECTION must match the fp32 reference almost exactly (one
flipped neighbor swaps a whole row into the mean; measured: f32r/tf32
scores -> 4e-2 rel err, FAIL the 2e-2 gate).  Scores therefore use a
3-pass bf16 hi/lo scheme: with h = hi + lo (both bf16), S ~ hi.hi +
hi.lo + lo.hi (lo.lo ~ 2^-18, dropped) gives ~17-bit effective operand
precision; a host-side simulation of the full pipeline at 16-bit operand
precision shows 0 flips / 1.7e-7 rel err.  The VALUE paths are cheap:
aggregation uses fp16 rows (11-bit mantissa: bf16-hi rows left ~1 flip
of state-divergence feedback at 2.1e-2; fp16 cuts that 8x -> 7.2e-3),
and each core keeps the fp32 momentum chain (myh) for every block it
updates.

Every core runs an IDENTICAL instruction stream; per-core differences
(which tail blocks, causal masks, per-row neighbor counts) live in input
DATA only.  Tail state sits at uniform addresses (myh slices 8/9, hTmT
slots); gathered tail rows land block-indexed in hrows / hT cols 8-15.

Blocks are processed tails-first, then repl blocks in DESCENDING index
order.  Causality makes this single-buffer safe: block b reads only
h^(r-1) of blocks j <= b, and the in-place update of block b happens
after every this-round reader of block b has run (they all sit earlier
in the order).

Per round, per block b (window W_b = (b+1)*128; tails padded to 12/16
chunks):
  scores  3 bf16 passes accumulated in fp32 PSUM over 8 d-chunks:
          lhsT in {hTh|hTl cols b, hTmh|hTml slot}, rhs in {hTh, hTl}.
          PSUM->SBUF drain: plain copies on ACT; only the diagonal
          128-chunk (tails: the padded last 4 chunks) gets the additive
          causal mask via a DVE scalar_tensor_tensor.
  top-k   nc.vector.max (top-8) [+ match_replace + max for K=16] -> th.
  mask    M01 = (S >= th) 0/1 fp16 -> PE-transposed (groups of 4 chunks).
  agg     msg = M01 @ hrows (fp16 rows; fp32 PSUM).
  update  u = (myh_b + msg*w2)*(gain*mix) + bias; gelu on ACT;
          myh_b = mom*myh_b + (1-mom)*gelu(u)   (split DVE/Pool/ACT).
  layout  (r<2) fp16(myh_b) -> hrows[b] (ACT cast); hi = bf16(myh_b)
          (DVE) and lo = myh_b - hi (Pool) are PE-transposed into
          hTh/hTl cols b (repl) or the hTmh/hTml slot (tails).
Round boundary (r<2): one AllGather per tail slot, fired right after
that slot's update (start of the round); back-DMAs + hi/lo casts + PE
transposes into hrows / hT cols 8-15 are emitted at the boundary and
complete while the next round's repl work proceeds.  The final round
stores all 10 slices' (h-x)*scale rows; the host picks each core's
owned blocks.

All scalar parameters (sigmoid/softplus of the inputs) are applied on
the host into small input tensors, so the device program depends only on
shapes.
"""

import math
import numpy as np

import concourse.bacc as bacc
import concourse.bass as bass
import concourse.mybir as mybir
import concourse.tile as tile
from concourse import bass_utils
from concourse.alu_op_type import AluOpType

F32 = mybir.dt.float32
BF16 = mybir.dt.bfloat16
FP16 = mybir.dt.float16
AF = mybir.ActivationFunctionType
BF16_NP = mybir.dt.np(BF16)
FP16_NP = mybir.dt.np(FP16)

NEG_MASK = -3.0e38  # additive causal mask value (bf16-representable)
NEG_CLAMP = -1.0e29  # threshold clamp: above mask, below any real score

K_SCHEDULE = (4, 8, 16)


class Cfg:
    def __init__(self, B=2, T=2048, D=1024, G=4, MR=8):
        self.B, self.T, self.D, self.G, self.MR = B, T, D, G, MR
        self.P = 128
        self.DC = D // 128            # d-chunks
        self.NBLK = T // 128          # row blocks per batch (16)
        self.NT = (self.NBLK - MR) // G   # tail slots per core (2)
        self.NS = MR + self.NT        # per-core myh slices (repl + tails)
        self.n_cores = B * G
        self.R = len(K_SCHEDULE)
        self.groups = [list(range(b * G, (b + 1) * G)) for b in range(B)]

    def tail_block(self, t, cc):
        return self.MR + t * self.G + cc

    def tail_w(self, t):
        """padded causal window (chunks) of tail slot t"""
        return self.MR + (t + 1) * self.G


def build_program(cfg: Cfg):
    """Build the single SPMD Bass/Tile program (identical on all cores)."""
    nc = bacc.Bacc(
        "TRN2", target_bir_lowering=False, debug=False,
        num_devices=cfg.n_cores,
    )
    P, D, T, DC, G, R = cfg.P, cfg.D, cfg.T, cfg.DC, cfg.G, cfg.R
    MR, NT, NS, NBLK = cfg.MR, cfg.NT, cfg.NS, cfg.NBLK

    # ---- I/O ----
    i_hTh = nc.dram_tensor("i_hTh", [P, DC * T], BF16, kind="ExternalInput")
    i_hTl = nc.dram_tensor("i_hTl", [P, DC * T], BF16, kind="ExternalInput")
    i_hrows = nc.dram_tensor("i_hrows", [P, NBLK * D], FP16, kind="ExternalInput")
    i_myh = nc.dram_tensor("i_myh", [P, NS * D], F32, kind="ExternalInput")
    i_hTmh = nc.dram_tensor("i_hTmh", [P, NT * D], BF16, kind="ExternalInput")
    i_hTml = nc.dram_tensor("i_hTml", [P, NT * D], BF16, kind="ExternalInput")
    i_dmsk = nc.dram_tensor("i_dmsk", [P, 128], BF16, kind="ExternalInput")
    i_tmsk = nc.dram_tensor("i_tmsk", [P, NT * G * 128], BF16, kind="ExternalInput")
    i_xs = nc.dram_tensor("i_xs", [P, NS * D], F32, kind="ExternalInput")
    i_gm = nc.dram_tensor("i_gm", [R, P, D], F32, kind="ExternalInput")
    i_bb = nc.dram_tensor("i_bb", [R, P, D], F32, kind="ExternalInput")
    i_w2 = nc.dram_tensor("i_w2", [P, R * NS], F32, kind="ExternalInput")
    # per-partition scalar params: col 0 = mom, 1 = s*(1-mom), 2 = s*mom, 3 = 1-mom
    i_sc = nc.dram_tensor("i_sc", [P, 4], F32, kind="ExternalInput")
    i_idb = nc.dram_tensor("i_idb", [P, 128], BF16, kind="ExternalInput")
    i_idh = nc.dram_tensor("i_idh", [P, 128], FP16, kind="ExternalInput")
    o_out = nc.dram_tensor("o_out", [NS, P, D], F32, kind="ExternalOutput")

    NH = D // 512  # 512-wide halves of D
    with tile.TileContext(nc) as tc:
        with (
            tc.tile_pool(name="const", bufs=1) as const,
            tc.tile_pool(name="work", bufs=2) as work,
            tc.tile_pool(name="psum", bufs=2, space="PSUM") as psum,
            tc.tile_pool(name="dram", bufs=1, space="DRAM") as dram,
        ):
            # ---- persistent state ----
            hTh = const.tile([P, DC * T], BF16, name="hTh")
            hTl = const.tile([P, DC * T], BF16, name="hTl")
            hrows = const.tile([P, NBLK * D], FP16, name="hrows")
            myh = const.tile([P, NS * D], F32, name="myh")
            hTmh = const.tile([P, NT * D], BF16, name="hTmh")
            hTml = const.tile([P, NT * D], BF16, name="hTml")
            dmsk = const.tile([P, 128], BF16, name="dmsk")
            tmsk = const.tile([P, NT * G * 128], BF16, name="tmsk")
            w2t = const.tile([P, R * NS], F32, name="w2t")
            sct = const.tile([P, 4], F32, name="sct")
            idb = const.tile([P, 128], BF16, name="idb")
            idh = const.tile([P, 128], FP16, name="idh")

            hThv = hTh.rearrange("p (c j) -> p c j", c=DC)
            hTlv = hTl.rearrange("p (c j) -> p c j", c=DC)

            # initial loads: consts + tail lhsT first on sync; bulk state
            # striped across the three DGE queues in first-use order.
            nc.sync.dma_start(idb[:], i_idb[:])
            nc.sync.dma_start(idh[:], i_idh[:])
            nc.sync.dma_start(sct[:], i_sc[:])
            nc.sync.dma_start(w2t[:], i_w2[:])
            nc.sync.dma_start(dmsk[:], i_dmsk[:])
            nc.sync.dma_start(tmsk[:], i_tmsk[:])
            nc.sync.dma_start(hTmh[:], i_hTmh[:])
            nc.sync.dma_start(hTml[:], i_hTml[:])
            iThv = i_hTh[:].rearrange("p (c j) -> p c j", c=DC)
            iTlv = i_hTl[:].rearrange("p (c j) -> p c j", c=DC)
            for dc in range(DC):
                eng = (nc.sync, nc.scalar, nc.gpsimd)[dc % 3]
                eng.dma_start(hThv[:, dc], iThv[:, dc])
                eng.dma_start(hTlv[:, dc], iTlv[:, dc])
            for b in range(NBLK):
                eng = (nc.sync, nc.scalar, nc.gpsimd)[b % 3]
                eng.dma_start(hrows[:, b * D:(b + 1) * D], i_hrows[:, b * D:(b + 1) * D])
            for b in range(NS):
                eng = (nc.scalar, nc.gpsimd)[b % 2]
                eng.dma_start(myh[:, b * D:(b + 1) * D], i_myh[:, b * D:(b + 1) * D])

            ap_mom = sct[:, 0:1]
            ap_s1m = sct[:, 1:2]
            ap_sm = sct[:, 2:3]
            ap_1m = sct[:, 3:4]

            ag_in = [[dram.tile([1, P, D], F32, name=f"ag_in{r}_{t}", tag=f"agi{r}_{t}")
                      for t in range(NT)] for r in range(R - 1)]
            ag_out = [[dram.tile([G, P, D], F32, name=f"ag_out{r}_{t}", tag=f"ago{r}_{t}")
                       for t in range(NT)] for r in range(R - 1)]

            def split_and_transpose(name, src_f32_ap, rows_dst, blk, tslot):
                """rows_dst (fp16 hrows slice, or None) = cast of the f32
                rows; score path: hi = bf16(rows), lo = rows - hi, both PE-
                transposed into hT cols blk (blk >= 0) or the hTmT slot
                tslot (blk < 0)."""
                if rows_dst is not None:
                    nc.scalar.copy(rows_dst, src_f32_ap)
                hi = work.tile([P, D], BF16, tag="hi_t", bufs=2, name=f"hi_{name}")
                nc.vector.tensor_copy(hi[:], src_f32_ap)
                lo = work.tile([P, D], BF16, tag="lo_t", bufs=2, name=f"lo_{name}")
                nc.gpsimd.tensor_sub(lo[:], src_f32_ap, hi[:])
                for half, srcs in ((0, hi[:]), (1, lo[:])):
                    for h_ in range(NH):
                        pt2 = psum.tile([P, 512], BF16, tag="ps_tp", name=f"pt2_{name}_{half}_{h_}")
                        for q in range(4):
                            dc = h_ * 4 + q
                            nc.tensor.transpose(
                                pt2[:, q * 128:(q + 1) * 128],
                                srcs[:, dc * 128:(dc + 1) * 128], idb[:])
                        if blk >= 0:
                            dstv = hThv if half == 0 else hTlv
                            dst = dstv[:, h_ * 4:(h_ + 1) * 4, blk * 128:(blk + 1) * 128]
                            nc.vector.tensor_copy(
                                dst, pt2[:].rearrange("p (c j) -> p c j", c=4))
                        else:
                            dstt = hTmh if half == 0 else hTml
                            nc.vector.tensor_copy(
                                dstt[:, tslot * D + h_ * 512: tslot * D + (h_ + 1) * 512],
                                pt2[:])

            def emit_scores(r, b, tslot, gmt, bbt):
                """Emit one block's score matmuls (PE only); drains deferred."""
                K = K_SCHEDULE[r]
                tail = tslot >= 0
                nchunk = cfg.tail_w(tslot) if tail else (b + 1)
                W = nchunk * 128
                name = f"{r}_t{tslot}" if tail else f"{r}_{b}"
                sc = work.tile([P, W], F32, tag="sc", bufs=2, name=f"sc_{name}")
                scr = None
                for w0 in range(0, W, 512):
                    n = min(512, W - w0)
                    ps = psum.tile([P, 512], F32, tag="ps_sc", bufs=2, name=f"ps_{name}_{w0}")
                    first = True
                    for lhsTt, rhst in ((0, 0), (0, 1), (1, 0)):
                        for dc in range(DC):
                            if tail:
                                lsrc = hTmh if lhsTt == 0 else hTml
                                l = lsrc[:, tslot * D + dc * 128: tslot * D + (dc + 1) * 128]
                            else:
                                lsrc = hTh if lhsTt == 0 else hTl
                                l = lsrc[:, dc * T + b * 128: dc * T + (b + 1) * 128]
                            rsrc = hTh if rhst == 0 else hTl
                            nc.tensor.matmul(
                                ps[:, :n], l,
                                rsrc[:, dc * T + w0: dc * T + w0 + n],
                                start=first, stop=(lhsTt == 1 and dc == DC - 1),
                            )
                            first = False
                    # drain PSUM -> SBUF immediately: plain copies on ACT,
                    # additive causal mask (diagonal / padded chunks) on DVE
                    if tail:
                        mwid = G * 128 if w0 + n == W else 0
                    else:
                        mwid = 128 if w0 + n == W else 0
                    if n > mwid:
                        nc.scalar.copy(sc[:, w0:w0 + n - mwid], ps[:, :n - mwid])
                    if mwid:
                        mk = tmsk[:, tslot * G * 128:(tslot + 1) * G * 128] if tail else dmsk[:]
                        nc.vector.scalar_tensor_tensor(
                            sc[:, w0 + n - mwid:w0 + n], ps[:, n - mwid:n], 1.0,
                            mk, AluOpType.mult, AluOpType.add)
                return dict(r=r, b=b, tslot=tslot, tail=tail, K=K, W=W,
                            nchunk=nchunk, name=name, sc=sc, scr=scr,
                            gmt=gmt, bbt=bbt)

            def emit_rest(ctx):
                """top-k, neighbor mask, aggregation, elementwise update and
                state propagation for a block whose scores are drained."""
                r, b, tslot, tail = ctx["r"], ctx["b"], ctx["tslot"], ctx["tail"]
                K, W, nchunk, name = ctx["K"], ctx["W"], ctx["nchunk"], ctx["name"]
                sc, gmt, bbt = ctx["sc"], ctx["gmt"], ctx["bbt"]
                msl = (MR + tslot) if tail else b
                # ---- top-K threshold ----
                mx = work.tile([P, 8], F32, tag="mx", name=f"mx_{name}")
                nc.vector.max(out=mx[:], in_=sc[:])
                if K <= 8:
                    th_src = mx[:, K - 1:K]
                else:
                    scr = work.tile([P, W], F32, tag="sc", bufs=2, name=f"scr_{name}")
                    nc.vector.match_replace(out=scr[:], in_to_replace=mx[:],
                                            in_values=sc[:], imm_value=NEG_MASK)
                    mx2 = work.tile([P, 8], F32, tag="mx2", name=f"mx2_{name}")
                    nc.vector.max(out=mx2[:], in_=scr[:])
                    th_src = mx2[:, K - 9:K - 8]
                th = work.tile([P, 1], F32, tag="th", name=f"th_{name}")
                nc.vector.tensor_scalar_max(th[:], th_src, NEG_CLAMP)
                # ---- 0/1 neighbor mask (fp16) ----
                m01 = work.tile([P, W], FP16, tag="m01", bufs=1, name=f"m01_{name}")
                nc.vector.tensor_scalar(m01[:], sc[:], th[:], None, AluOpType.is_ge)
                # ---- transpose mask in groups of 4 chunks ----
                ngrp = (nchunk + 3) // 4
                mts = []
                for gi in range(ngrp):
                    lo_c = gi * 4
                    hi_c = min(lo_c + 4, nchunk)
                    pt = psum.tile([P, 512], FP16, tag="ps_tr", name=f"pt_{name}_{gi}")
                    for jc in range(lo_c, hi_c):
                        nc.tensor.transpose(pt[:, (jc - lo_c) * 128:(jc - lo_c + 1) * 128],
                                            m01[:, jc * 128:(jc + 1) * 128], idh[:])
                    mt = work.tile([P, 512], FP16, tag="mt", bufs=5, name=f"mt_{name}_{gi}")
                    nc.vector.tensor_copy(mt[:, 0:(hi_c - lo_c) * 128], pt[:, 0:(hi_c - lo_c) * 128])
                    mts.append(mt)
                # ---- aggregate: msg = M01 @ hrows (fp16 rows) ----
                pss = []
                for h_ in range(NH):
                    pa = psum.tile([P, 512], F32, tag="ps_ag", bufs=2, name=f"pa_{name}_{h_}")
                    for jc in range(nchunk):
                        nc.tensor.matmul(
                            pa[:],
                            mts[jc // 4][:, (jc % 4) * 128:(jc % 4 + 1) * 128],
                            hrows[:, jc * D + h_ * 512: jc * D + h_ * 512 + 512],
                            start=(jc == 0), stop=(jc == nchunk - 1))
                    pss.append(pa)
                # ---- elementwise update ----
                w2ap = w2t[:, r * NS + msl: r * NS + msl + 1]
                for h_ in range(NH):
                    sl = slice(msl * D + h_ * 512, msl * D + (h_ + 1) * 512)
                    hsl = slice(h_ * 512, (h_ + 1) * 512)
                    t1 = work.tile([P, 512], F32, tag="t1", name=f"t1_{name}_{h_}")
                    nc.vector.scalar_tensor_tensor(
                        t1[:], pss[h_][:], w2ap, myh[:, sl],
                        AluOpType.mult, AluOpType.add)
                    nc.gpsimd.tensor_mul(t1[:], t1[:], gmt[:, hsl])
                    nc.gpsimd.tensor_add(t1[:], t1[:], bbt[:, hsl])
                    gl = work.tile([P, 512], F32, tag="gl", name=f"gl_{name}_{h_}")
                    nc.scalar.activation(gl[:], t1[:], AF.Gelu)
                    if r < R - 1:
                        nc.vector.tensor_scalar_mul(gl[:], gl[:], ap_1m)
                        # in-place: myh = mom*myh + (1-mom)*gelu
                        nc.vector.scalar_tensor_tensor(
                            myh[:, sl], myh[:, sl], ap_mom, gl[:],
                            AluOpType.mult, AluOpType.add)
                    else:
                        xst = work.tile([P, 512], F32, tag="xst", bufs=1, name=f"xst_{name}_{h_}")
                        nc.sync.dma_start(xst[:], i_xs[:, sl])
                        # gl <- s*(1-mom)*gelu - s*x
                        nc.vector.scalar_tensor_tensor(
                            gl[:], gl[:], ap_s1m, xst[:],
                            AluOpType.mult, AluOpType.subtract)
                        # t1 <- s*mom*h + gl
                        nc.vector.scalar_tensor_tensor(
                            t1[:], myh[:, sl], ap_sm, gl[:],
                            AluOpType.mult, AluOpType.add)
                        nc.sync.dma_start(o_out[msl, :, hsl], t1[:])
                # ---- propagate updated rows (hi/lo + transposed layouts) ----
                if r < R - 1:
                    rows_ap = myh[:, msl * D:(msl + 1) * D]
                    if tail:
                        # hi rows of the own tail land via the AllGather back
                        # path (uniform for all 4 gathered blocks); here only
                        # the transposed lhsT + the collective payload.
                        split_and_transpose(name, rows_ap, None, -1, tslot)
                        nc.sync.dma_start(ag_in[r][tslot][0], rows_ap)
                        nc.gpsimd.collective_compute(
                            "AllGather", AluOpType.bypass, replica_groups=cfg.groups,
                            ins=[ag_in[r][tslot].opt()], outs=[ag_out[r][tslot].opt()])
                    else:
                        split_and_transpose(name, rows_ap, hrows[:, b * D:(b + 1) * D], b, -1)

            def emit_boundary(r, t):
                """Land tail slot t's gathered rows in hrows + hT columns."""
                for i in range(G):
                    blk = MR + t * G + i
                    tr = work.tile([P, D], F32, tag="trow", bufs=2, name=f"tr_{r}_{t}_{i}")
                    nc.sync.dma_start(tr[:], ag_out[r][t][i])
                    split_and_transpose(
                        f"bnd{r}_{t}_{i}", tr[:],
                        hrows[:, blk * D:(blk + 1) * D], blk, -1)

            for r in range(R):
                # per-round small loads on the ACT HWDGE queue
                gmt = work.tile([P, D], F32, tag="gmt", bufs=1, name=f"gmt{r}")
                nc.scalar.dma_start(gmt[:], i_gm[r])
                bbt = work.tile([P, D], F32, tag="bbt", bufs=1, name=f"bbt{r}")
                nc.scalar.dma_start(bbt[:], i_bb[r])

                # sequential emission (the Tile scheduler overlaps via its
                # dependency lookahead; manual interleaving measured slower)
                seq = [("t", t) for t in range(NT)] + [("b", b) for b in range(MR - 1, -1, -1)]
                for kind, idx in seq:
                    if r > 0 and kind == "t" and idx == 1:
                        emit_boundary(r - 1, 1)
                    if kind == "t":
                        ctx = emit_scores(r, -1, idx, gmt, bbt)
                    else:
                        ctx = emit_scores(r, idx, -1, gmt, bbt)
                    emit_rest(ctx)
                if r < R - 1:
                    emit_boundary(r, 0)

    nc.compile()
    return nc


# ------------------------------------------------------------------
# Host side
# ------------------------------------------------------------------

def _sigmoid(v):
    return 1.0 / (1.0 + math.exp(-float(v)))


def prep_inputs(cfg: Cfg, x, gain, bias, log_mix, log_momentum, log_scale):
    """Build the per-core input maps (numpy)."""
    P, D, T, DC, G, R = cfg.P, cfg.D, cfg.T, cfg.DC, cfg.G, cfg.R
    MR, NT, NS, NBLK = cfg.MR, cfg.NT, cfg.NS, cfg.NBLK
    x = np.asarray(x, np.float32)
    gain = np.asarray(gain, np.float32)
    bias = np.asarray(bias, np.float32)
    mix = np.array([_sigmoid(v) for v in np.asarray(log_mix, np.float32)], np.float64)
    mom = _sigmoid(log_momentum)
    s = math.log1p(math.exp(float(log_scale))) + 0.01

    gm = np.ascontiguousarray(
        np.broadcast_to((gain * mix[:, None].astype(np.float32)).astype(np.float32)[:, None, :], (R, P, D)))
    bb = np.ascontiguousarray(np.broadcast_to(bias[:, None, :], (R, P, D)))
    scl = np.zeros((P, 4), np.float32)
    scl[:, 0] = mom
    scl[:, 1] = s * (1.0 - mom)
    scl[:, 2] = s * mom
    scl[:, 3] = 1.0 - mom
    idb = np.eye(128, dtype=BF16_NP)
    idh = np.eye(128, dtype=FP16_NP)
    p_ = np.arange(P)
    dmsk = np.where(np.arange(128)[None, :] <= p_[:, None], 0.0, NEG_MASK).astype(BF16_NP)

    def transposed_cols(rows_bf):  # [nb,128,D] bf16 -> [128, nb*D] chunk-T
        nb = rows_bf.shape[0]
        return np.ascontiguousarray(
            rows_bf.reshape(nb, 128, DC, 128).transpose(3, 0, 2, 1)).reshape(128, nb * D)

    in_maps = []
    for c in range(cfg.n_cores):
        bidx, cc = divmod(c, G)
        h0 = x[bidx]                       # [T, D]
        hblk = h0.reshape(NBLK, 128, D)
        hi_f = h0.astype(BF16_NP)
        lo_f = (h0 - hi_f.astype(np.float32)).astype(BF16_NP)
        hTh0 = np.ascontiguousarray(
            hi_f.T.reshape(DC, 128, T).transpose(1, 0, 2)).reshape(128, DC * T)
        hTl0 = np.ascontiguousarray(
            lo_f.T.reshape(DC, 128, T).transpose(1, 0, 2)).reshape(128, DC * T)
        hrows0 = np.ascontiguousarray(
            h0.astype(FP16_NP).reshape(NBLK, 128, D).transpose(1, 0, 2)).reshape(128, NBLK * D)
        tblocks = [cfg.tail_block(t, cc) for t in range(NT)]
        blocks = list(range(MR)) + tblocks  # myh slice -> block
        myh0 = np.ascontiguousarray(hblk[blocks].transpose(1, 0, 2)).reshape(128, NS * D)
        hTmh0 = transposed_cols(hi_f.reshape(NBLK, 128, D)[tblocks])
        hTml0 = transposed_cols(lo_f.reshape(NBLK, 128, D)[tblocks])
        # tail masks: slot t's masked region = its padded window's last
        # G*128 cols; allow iff absolute j <= blk*128 + p
        tmsk = np.zeros((P, NT * G * 128), BF16_NP)
        for t in range(NT):
            jj = np.arange(G * 128)
            tmsk[:, t * G * 128:(t + 1) * G * 128] = np.where(
                jj[None, :] <= cc * 128 + p_[:, None], 0.0, NEG_MASK).astype(BF16_NP)
        w2 = np.zeros((P, R * NS), np.float32)
        for r in range(R):
            for sl in range(NS):
                blk = blocks[sl]
                cnt = np.minimum(blk * 128 + p_ + 1, K_SCHEDULE[r])
                w2[:, r * NS + sl] = ((1.0 - mix[r]) / (mix[r] * cnt)).astype(np.float32)
        in_maps.append({
            "i_hTh": hTh0, "i_hTl": hTl0, "i_hrows": hrows0, "i_myh": myh0,
            "i_hTmh": hTmh0, "i_hTml": hTml0,
            "i_dmsk": dmsk, "i_tmsk": tmsk,
            "i_xs": (s * myh0.astype(np.float64)).astype(np.float32),
            "i_gm": gm, "i_bb": bb, "i_w2": w2, "i_sc": scl,
            "i_idb": idb, "i_idh": idh,
        })
    return in_maps


def assemble_output(cfg: Cfg, results, dtype=np.float32):
    """results: list (per core) of {'o_out': [NS,128,D]} -> full [B,T,D]."""
    out = np.zeros((cfg.B, cfg.T, cfg.D), dtype)
    for bidx in range(cfg.B):
        base = bidx * cfg.G
        o0 = results[base]["o_out"]
        for blk in range(cfg.MR):
            out[bidx, blk * 128:(blk + 1) * 128] = o0[blk]
        for cc in range(cfg.G):
            for t in range(cfg.NT):
                blk = cfg.tail_block(t, cc)
                out[bidx, blk * 128:(blk + 1) * 128] = results[base + cc]["o_out"][cfg.MR + t]
    return out


_PROGRAM_CACHE = {}


def _get_program(cfg: Cfg):
    key = (cfg.B, cfg.T, cfg.D, cfg.G, cfg.MR)
    if key not in _PROGRAM_CACHE:
        _PROGRAM_CACHE[key] = build_program(cfg)
    return _PROGRAM_CACHE[key]


def run(cfg: Cfg, inputs: dict, trace: bool = False):
    nc = _get_program(cfg)
    in_maps = prep_inputs(cfg, **inputs)
    res = bass_utils.run_bass_kernel_spmd(
        nc, in_maps, list(range(cfg.n_cores)), trace=trace)
    out = assemble_output(cfg, res.results)
    return out, res


def kernel(**inputs) -> np.ndarray:
    cfg = Cfg()  # B=2, T=2048, D=1024, 8 cores
    out, _ = run(cfg, inputs)
    return out.astype(np.float32)


# revision 22
# speedup vs baseline: 1.0325x; 1.0325x over previous
"""Trainium2 Bass kernel for nn_DGN6 (gnn_message_passing).

Reference computation (per batch element, 3 rounds with K = 4, 8, 16):
    S = h @ h.T; causal top-K neighbors per row; msg = masked mean of
    neighbor rows; h = mom*h + (1-mom)*gelu((mix*h + (1-mix)*msg)*gain + bias)
Output: (h - x) * scale.

Distribution: data-parallel over B (2 batches); within a batch, PARTIAL
REPLICATION over its 4 cores.  The leading 8 row-blocks (of 16 x 128
rows) are updated identically on every core of the group ("repl" blocks;
their inputs are causally local, so they need NO communication).  Blocks
8-15 are sharded as two tail slots per core (core cc owns blocks 8+cc
and 12+cc) and exchanged between rounds with two pipelined f32
AllGathers (2MB out each), which mostly hide behind the replicated
compute.  This trades redundant FLOPs for a large cut in collective
cost, which dominates the roofline here.

Top-k SELECTION must match the fp32 reference almost exactly (one
flipped neighbor swaps a whole row into the mean; measured: f32r/tf32
scores -> 4e-2 rel err, FAIL the 2e-2 gate).  Scores therefore use a
3-pass bf16 hi/lo scheme: with h = hi + lo (both bf16), S ~ hi.hi +
hi.lo + lo.hi (lo.lo ~ 2^-18, dropped) gives ~17-bit effective operand
precision; a host-side simulation of the full pipeline at 16-bit operand
precision shows 0 flips / 1.7e-7 rel err.  The VALUE paths are cheap:
aggregation uses fp16 rows (11-bit mantissa: bf16-hi rows left ~1 flip
of state-divergence feedback at 2.1e-2; fp16 cuts that 8x -> 7.2e-3),
and each core keeps the fp32 momentum chain (myh) for every block it
updates.

Every core runs an IDENTICAL instruction stream; per-core differences
(which tail blocks, causal masks, per-row neighbor counts) live in input
DATA only.  Tail state sits at uniform addresses (myh slices 8/9, hTmT
slots); gathered tail rows land block-indexed in hrows / hT cols 8-15.

Blocks are processed tails-first, then repl blocks in DESCENDING index
order.  Causality makes this single-buffer safe: block b reads only
h^(r-1) of blocks j <= b, and the in-place update of block b happens
after every this-round reader of block b has run (they all sit earlier
in the order).

Per round, per block b (window W_b = (b+1)*128; tails padded to 12/16
chunks):
  scores  3 bf16 passes accumulated in fp32 PSUM over 8 d-chunks:
          lhsT in {hTh|hTl cols b, hTmh|hTml slot}, rhs in {hTh, hTl}.
          PSUM->SBUF drain: plain copies on ACT; only the diagonal
          128-chunk (tails: the padded last 4 chunks) gets the additive
          causal mask via a DVE scalar_tensor_tensor.
  top-k   nc.vector.max (top-8) [+ match_replace + max for K=16] -> th.
  mask    M01 = (S >= th) 0/1 fp16 -> PE-transposed (groups of 4 chunks).
  agg     msg = M01 @ hrows (fp16 rows; fp32 PSUM).
  update  u = (myh_b + msg*w2)*(gain*mix) + bias; gelu on ACT;
          myh_b = mom*myh_b + (1-mom)*gelu(u)   (split DVE/Pool/ACT).
  layout  (r<2) fp16(myh_b) -> hrows[b] (ACT cast); hi = bf16(myh_b)
          (DVE) and lo = myh_b - hi (Pool) are PE-transposed into
          hTh/hTl cols b (repl) or the hTmh/hTml slot (tails).
Round boundary (r<2): one AllGather per tail slot, fired right after
that slot's update (start of the round); back-DMAs + hi/lo casts + PE
transposes into hrows / hT cols 8-15 are emitted at the boundary and
complete while the next round's repl work proceeds.  The final round
stores all 10 slices' (h-x)*scale rows; the host picks each core's
owned blocks.

All scalar parameters (sigmoid/softplus of the inputs) are applied on
the host into small input tensors, so the device program depends only on
shapes.
"""

import math
import numpy as np

import concourse.bacc as bacc
import concourse.bass as bass
import concourse.mybir as mybir
import concourse.tile as tile
from concourse import bass_utils
from concourse.alu_op_type import AluOpType

F32 = mybir.dt.float32
BF16 = mybir.dt.bfloat16
FP16 = mybir.dt.float16
AF = mybir.ActivationFunctionType
BF16_NP = mybir.dt.np(BF16)
FP16_NP = mybir.dt.np(FP16)

NEG_MASK = -3.0e38  # additive causal mask value (bf16-representable)
NEG_CLAMP = -1.0e29  # threshold clamp: above mask, below any real score

K_SCHEDULE = (4, 8, 16)


class Cfg:
    def __init__(self, B=2, T=2048, D=1024, G=4, MR=8):
        self.B, self.T, self.D, self.G, self.MR = B, T, D, G, MR
        self.P = 128
        self.DC = D // 128            # d-chunks
        self.NBLK = T // 128          # row blocks per batch (16)
        self.NT = (self.NBLK - MR) // G   # tail slots per core (2)
        self.NS = MR + self.NT        # per-core myh slices (repl + tails)
        self.n_cores = B * G
        self.R = len(K_SCHEDULE)
        self.groups = [list(range(b * G, (b + 1) * G)) for b in range(B)]

    def tail_block(self, t, cc):
        return self.MR + t * self.G + cc

    def tail_w(self, t):
        """padded causal window (chunks) of tail slot t"""
        return self.MR + (t + 1) * self.G


def build_program(cfg: Cfg):
    """Build the single SPMD Bass/Tile program (identical on all cores)."""
    nc = bacc.Bacc(
        "TRN2", target_bir_lowering=False, debug=False,
        num_devices=cfg.n_cores,
    )
    P, D, T, DC, G, R = cfg.P, cfg.D, cfg.T, cfg.DC, cfg.G, cfg.R
    MR, NT, NS, NBLK = cfg.MR, cfg.NT, cfg.NS, cfg.NBLK

    # ---- I/O ----
    i_hTh = nc.dram_tensor("i_hTh", [P, DC * T], BF16, kind="ExternalInput")
    i_hTl = nc.dram_tensor("i_hTl", [P, DC * T], BF16, kind="ExternalInput")
    i_hrows = nc.dram_tensor("i_hrows", [P, NBLK * D], FP16, kind="ExternalInput")
    i_myh = nc.dram_tensor("i_myh", [P, NS * D], F32, kind="ExternalInput")
    i_hTmh = nc.dram_tensor("i_hTmh", [P, NT * D], BF16, kind="ExternalInput")
    i_hTml = nc.dram_tensor("i_hTml", [P, NT * D], BF16, kind="ExternalInput")
    i_dmsk = nc.dram_tensor("i_dmsk", [P, 128], BF16, kind="ExternalInput")
    i_tmsk = nc.dram_tensor("i_tmsk", [P, NT * G * 128], BF16, kind="ExternalInput")
    i_xs = nc.dram_tensor("i_xs", [P, NS * D], F32, kind="ExternalInput")
    i_gm = nc.dram_tensor("i_gm", [R, P, D], F32, kind="ExternalInput")
    i_bb = nc.dram_tensor("i_bb", [R, P, D], F32, kind="ExternalInput")
    i_w2 = nc.dram_tensor("i_w2", [P, R * NS], F32, kind="ExternalInput")
    # per-partition scalar params: col 0 = mom, 1 = s*(1-mom), 2 = s*mom, 3 = 1-mom
    i_sc = nc.dram_tensor("i_sc", [P, 4], F32, kind="ExternalInput")
    i_idb = nc.dram_tensor("i_idb", [P, 128], BF16, kind="ExternalInput")
    i_idh = nc.dram_tensor("i_idh", [P, 128], FP16, kind="ExternalInput")
    o_out = nc.dram_tensor("o_out", [NS, P, D], F32, kind="ExternalOutput")

    NH = D // 512  # 512-wide halves of D
    with tile.TileContext(nc) as tc:
        with (
            tc.tile_pool(name="const", bufs=1) as const,
            tc.tile_pool(name="work", bufs=2) as work,
            tc.tile_pool(name="psum", bufs=2, space="PSUM") as psum,
            tc.tile_pool(name="dram", bufs=1, space="DRAM") as dram,
        ):
            # ---- persistent state ----
            hTh = const.tile([P, DC * T], BF16, name="hTh")
            hTl = const.tile([P, DC * T], BF16, name="hTl")
            hrows = const.tile([P, NBLK * D], FP16, name="hrows")
            myh = const.tile([P, NS * D], F32, name="myh")
            hTmh = const.tile([P, NT * D], BF16, name="hTmh")
            hTml = const.tile([P, NT * D], BF16, name="hTml")
            dmsk = const.tile([P, 128], BF16, name="dmsk")
            tmsk = const.tile([P, NT * G * 128], BF16, name="tmsk")
            w2t = const.tile([P, R * NS], F32, name="w2t")
            sct = const.tile([P, 4], F32, name="sct")
            idb = const.tile([P, 128], BF16, name="idb")
            idh = const.tile([P, 128], FP16, name="idh")

            hThv = hTh.rearrange("p (c j) -> p c j", c=DC)
            hTlv = hTl.rearrange("p (c j) -> p c j", c=DC)

            # initial loads: consts + tail lhsT first on sync; bulk state
            # striped across the three DGE queues in first-use order.
            nc.sync.dma_start(idb[:], i_idb[:])
            nc.sync.dma_start(idh[:], i_idh[:])
            nc.sync.dma_start(sct[:], i_sc[:])
            nc.sync.dma_start(w2t[:], i_w2[:])
            nc.sync.dma_start(dmsk[:], i_dmsk[:])
            nc.sync.dma_start(tmsk[:], i_tmsk[:])
            nc.sync.dma_start(hTmh[:], i_hTmh[:])
            nc.sync.dma_start(hTml[:], i_hTml[:])
            iThv = i_hTh[:].rearrange("p (c j) -> p c j", c=DC)
            iTlv = i_hTl[:].rearrange("p (c j) -> p c j", c=DC)
            for dc in range(DC):
                eng = (nc.sync, nc.scalar, nc.gpsimd)[dc % 3]
                eng.dma_start(hThv[:, dc], iThv[:, dc])
                eng.dma_start(hTlv[:, dc], iTlv[:, dc])
            for b in range(NBLK):
                eng = (nc.sync, nc.scalar, nc.gpsimd)[b % 3]
                eng.dma_start(hrows[:, b * D:(b + 1) * D], i_hrows[:, b * D:(b + 1) * D])
            for b in range(NS):
                eng = (nc.scalar, nc.gpsimd)[b % 2]
                eng.dma_start(myh[:, b * D:(b + 1) * D], i_myh[:, b * D:(b + 1) * D])

            ap_mom = sct[:, 0:1]
            ap_s1m = sct[:, 1:2]
            ap_sm = sct[:, 2:3]
            ap_1m = sct[:, 3:4]

            ag_in = [[dram.tile([1, P, D], F32, name=f"ag_in{r}_{t}", tag=f"agi{r}_{t}")
                      for t in range(NT)] for r in range(R - 1)]
            ag_out = [[dram.tile([G, P, D], F32, name=f"ag_out{r}_{t}", tag=f"ago{r}_{t}")
                       for t in range(NT)] for r in range(R - 1)]

            def split_and_transpose(name, src_f32_ap, rows_dst, blk, tslot):
                """rows_dst (fp16 hrows slice, or None) = cast of the f32
                rows; score path: hi = bf16(rows), lo = rows - hi, both PE-
                transposed into hT cols blk (blk >= 0) or the hTmT slot
                tslot (blk < 0)."""
                if rows_dst is not None:
                    nc.scalar.copy(rows_dst, src_f32_ap)
                hi = work.tile([P, D], BF16, tag="hi_t", bufs=2, name=f"hi_{name}")
                nc.vector.tensor_copy(hi[:], src_f32_ap)
                lo = work.tile([P, D], BF16, tag="lo_t", bufs=2, name=f"lo_{name}")
                nc.gpsimd.tensor_sub(lo[:], src_f32_ap, hi[:])
                for half, srcs in ((0, hi[:]), (1, lo[:])):
                    for h_ in range(NH):
                        pt2 = psum.tile([P, 512], BF16, tag="ps_tp", name=f"pt2_{name}_{half}_{h_}")
                        for q in range(4):
                            dc = h_ * 4 + q
                            nc.tensor.transpose(
                                pt2[:, q * 128:(q + 1) * 128],
                                srcs[:, dc * 128:(dc + 1) * 128], idb[:])
                        if blk >= 0:
                            dstv = hThv if half == 0 else hTlv
                            dst = dstv[:, h_ * 4:(h_ + 1) * 4, blk * 128:(blk + 1) * 128]
                            nc.vector.tensor_copy(
                                dst, pt2[:].rearrange("p (c j) -> p c j", c=4))
                        else:
                            dstt = hTmh if half == 0 else hTml
                            nc.vector.tensor_copy(
                                dstt[:, tslot * D + h_ * 512: tslot * D + (h_ + 1) * 512],
                                pt2[:])

            def emit_scores(r, b, tslot, gmt, bbt):
                """Emit one block's score matmuls (PE only); drains deferred."""
                K = K_SCHEDULE[r]
                tail = tslot >= 0
                nchunk = cfg.tail_w(tslot) if tail else (b + 1)
                W = nchunk * 128
                name = f"{r}_t{tslot}" if tail else f"{r}_{b}"
                sc = work.tile([P, W], F32, tag="sc", bufs=2, name=f"sc_{name}")
                scr = None
                for w0 in range(0, W, 512):
                    n = min(512, W - w0)
                    ps = psum.tile([P, 512], F32, tag="ps_sc", bufs=2, name=f"ps_{name}_{w0}")
                    first = True
                    for lhsTt, rhst in ((0, 0), (0, 1), (1, 0)):
                        for dc in range(DC):
                            if tail:
                                lsrc = hTmh if lhsTt == 0 else hTml
                                l = lsrc[:, tslot * D + dc * 128: tslot * D + (dc + 1) * 128]
                            else:
                                lsrc = hTh if lhsTt == 0 else hTl
                                l = lsrc[:, dc * T + b * 128: dc * T + (b + 1) * 128]
                            rsrc = hTh if rhst == 0 else hTl
                            nc.tensor.matmul(
                                ps[:, :n], l,
                                rsrc[:, dc * T + w0: dc * T + w0 + n],
                                start=first, stop=(lhsTt == 1 and dc == DC - 1),
                            )
                            first = False
                    # drain PSUM -> SBUF immediately: plain copies on ACT,
                    # additive causal mask (diagonal / padded chunks) on DVE
                    if tail:
                        mwid = G * 128 if w0 + n == W else 0
                    else:
                        mwid = 128 if w0 + n == W else 0
                    if n > mwid:
                        nc.scalar.copy(sc[:, w0:w0 + n - mwid], ps[:, :n - mwid])
                    if mwid:
                        mk = tmsk[:, tslot * G * 128:(tslot + 1) * G * 128] if tail else dmsk[:]
                        nc.vector.scalar_tensor_tensor(
                            sc[:, w0 + n - mwid:w0 + n], ps[:, n - mwid:n], 1.0,
                            mk, AluOpType.mult, AluOpType.add)
                return dict(r=r, b=b, tslot=tslot, tail=tail, K=K, W=W,
                            nchunk=nchunk, name=name, sc=sc, scr=scr,
                            gmt=gmt, bbt=bbt)

            def emit_rest(ctx):
                """top-k, neighbor mask, aggregation, elementwise update and
                state propagation for a block whose scores are drained."""
                r, b, tslot, tail = ctx["r"], ctx["b"], ctx["tslot"], ctx["tail"]
                K, W, nchunk, name = ctx["K"], ctx["W"], ctx["nchunk"], ctx["name"]
                sc, gmt, bbt = ctx["sc"], ctx["gmt"], ctx["bbt"]
                msl = (MR + tslot) if tail else b
                # ---- top-K threshold ----
                mx = work.tile([P, 8], F32, tag="mx", name=f"mx_{name}")
                nc.vector.max(out=mx[:], in_=sc[:])
                if K <= 8:
                    th_src = mx[:, K - 1:K]
                else:
                    scr = work.tile([P, W], F32, tag="sc", bufs=2, name=f"scr_{name}")
                    nc.vector.match_replace(out=scr[:], in_to_replace=mx[:],
                                            in_values=sc[:], imm_value=NEG_MASK)
                    mx2 = work.tile([P, 8], F32, tag="mx2", name=f"mx2_{name}")
                    nc.vector.max(out=mx2[:], in_=scr[:])
                    th_src = mx2[:, K - 9:K - 8]
                th = work.tile([P, 1], F32, tag="th", name=f"th_{name}")
                nc.vector.tensor_scalar_max(th[:], th_src, NEG_CLAMP)
                # ---- 0/1 neighbor mask (fp16) ----
                m01 = work.tile([P, W], FP16, tag="m01", bufs=1, name=f"m01_{name}")
                nc.vector.tensor_scalar(m01[:], sc[:], th[:], None, AluOpType.is_ge)
                # ---- transpose mask in groups of 4 chunks ----
                ngrp = (nchunk + 3) // 4
                mts = []
                for gi in range(ngrp):
                    lo_c = gi * 4
                    hi_c = min(lo_c + 4, nchunk)
                    pt = psum.tile([P, 512], FP16, tag="ps_tr", name=f"pt_{name}_{gi}")
                    for jc in range(lo_c, hi_c):
                        nc.tensor.transpose(pt[:, (jc - lo_c) * 128:(jc - lo_c + 1) * 128],
                                            m01[:, jc * 128:(jc + 1) * 128], idh[:])
                    mt = work.tile([P, 512], FP16, tag="mt", bufs=5, name=f"mt_{name}_{gi}")
                    nc.vector.tensor_copy(mt[:, 0:(hi_c - lo_c) * 128], pt[:, 0:(hi_c - lo_c) * 128])
                    mts.append(mt)
                # ---- aggregate: msg = M01 @ hrows (fp16 rows) ----
                pss = []
                for h_ in range(NH):
                    pa = psum.tile([P, 512], F32, tag="ps_ag", bufs=2, name=f"pa_{name}_{h_}")
                    for jc in range(nchunk):
                        nc.tensor.matmul(
                            pa[:],
                            mts[jc // 4][:, (jc % 4) * 128:(jc % 4 + 1) * 128],
                            hrows[:, jc * D + h_ * 512: jc * D + h_ * 512 + 512],
                            start=(jc == 0), stop=(jc == nchunk - 1))
                    pss.append(pa)
                # ---- elementwise update ----
                w2ap = w2t[:, r * NS + msl: r * NS + msl + 1]
                for h_ in range(NH):
                    sl = slice(msl * D + h_ * 512, msl * D + (h_ + 1) * 512)
                    hsl = slice(h_ * 512, (h_ + 1) * 512)
                    t1 = work.tile([P, 512], F32, tag="t1", name=f"t1_{name}_{h_}")
                    nc.vector.scalar_tensor_tensor(
                        t1[:], pss[h_][:], w2ap, myh[:, sl],
                        AluOpType.mult, AluOpType.add)
                    nc.gpsimd.tensor_mul(t1[:], t1[:], gmt[:, hsl])
                    nc.gpsimd.tensor_add(t1[:], t1[:], bbt[:, hsl])
                    gl = work.tile([P, 512], F32, tag="gl", name=f"gl_{name}_{h_}")
                    nc.scalar.activation(gl[:], t1[:], AF.Gelu)
                    if r < R - 1:
                        nc.vector.tensor_scalar_mul(gl[:], gl[:], ap_1m)
                        # in-place: myh = mom*myh + (1-mom)*gelu
                        nc.vector.scalar_tensor_tensor(
                            myh[:, sl], myh[:, sl], ap_mom, gl[:],
                            AluOpType.mult, AluOpType.add)
                    else:
                        xst = work.tile([P, 512], F32, tag="xst", bufs=1, name=f"xst_{name}_{h_}")
                        nc.sync.dma_start(xst[:], i_xs[:, sl])
                        # gl <- s*(1-mom)*gelu - s*x
                        nc.vector.scalar_tensor_tensor(
                            gl[:], gl[:], ap_s1m, xst[:],
                            AluOpType.mult, AluOpType.subtract)
                        # t1 <- s*mom*h + gl
                        nc.vector.scalar_tensor_tensor(
                            t1[:], myh[:, sl], ap_sm, gl[:],
                            AluOpType.mult, AluOpType.add)
                        nc.sync.dma_start(o_out[msl, :, hsl], t1[:])
                # ---- propagate updated rows (hi/lo + transposed layouts) ----
                if r < R - 1:
                    rows_ap = myh[:, msl * D:(msl + 1) * D]
                    if tail:
                        # hi rows of the own tail land via the AllGather back
                        # path (uniform for all 4 gathered blocks); here only
                        # the transposed lhsT + the collective payload.
                        split_and_transpose(name, rows_ap, None, -1, tslot)
                        nc.sync.dma_start(ag_in[r][tslot][0], rows_ap)
                        nc.gpsimd.collective_compute(
                            "AllGather", AluOpType.bypass, replica_groups=cfg.groups,
                            ins=[ag_in[r][tslot].opt()], outs=[ag_out[r][tslot].opt()])
                    else:
                        split_and_transpose(name, rows_ap, hrows[:, b * D:(b + 1) * D], b, -1)

            def emit_boundary(r, t):
                """Land tail slot t's gathered rows in hrows + hT columns."""
                for i in range(G):
                    blk = MR + t * G + i
                    tr = work.tile([P, D], F32, tag="trow", bufs=2, name=f"tr_{r}_{t}_{i}")
                    nc.sync.dma_start(tr[:], ag_out[r][t][i])
                    split_and_transpose(
                        f"bnd{r}_{t}_{i}", tr[:],
                        hrows[:, blk * D:(blk + 1) * D], blk, -1)

            for r in range(R):
                # per-round small loads on the ACT HWDGE queue
                gmt = work.tile([P, D], F32, tag="gmt", bufs=1, name=f"gmt{r}")
                nc.scalar.dma_start(gmt[:], i_gm[r])
                bbt = work.tile([P, D], F32, tag="bbt", bufs=1, name=f"bbt{r}")
                nc.scalar.dma_start(bbt[:], i_bb[r])

                # sequential emission (the Tile scheduler overlaps via its
                # dependency lookahead; manual interleaving measured slower)
                seq = [("t", t) for t in range(NT)] + [("b", b) for b in range(MR - 1, -1, -1)]
                for kind, idx in seq:
                    if kind == "t":
                        ctx = emit_scores(r, -1, idx, gmt, bbt)
                    else:
                        ctx = emit_scores(r, idx, -1, gmt, bbt)
                    emit_rest(ctx)
                if r < R - 1:
                    for t in range(NT):
                        emit_boundary(r, t)

    nc.compile()
    return nc


# ------------------------------------------------------------------
# Host side
# ------------------------------------------------------------------

def _sigmoid(v):
    return 1.0 / (1.0 + math.exp(-float(v)))


def prep_inputs(cfg: Cfg, x, gain, bias, log_mix, log_momentum, log_scale):
    """Build the per-core input maps (numpy)."""
    P, D, T, DC, G, R = cfg.P, cfg.D, cfg.T, cfg.DC, cfg.G, cfg.R
    MR, NT, NS, NBLK = cfg.MR, cfg.NT, cfg.NS, cfg.NBLK
    x = np.asarray(x, np.float32)
    gain = np.asarray(gain, np.float32)
    bias = np.asarray(bias, np.float32)
    mix = np.array([_sigmoid(v) for v in np.asarray(log_mix, np.float32)], np.float64)
    mom = _sigmoid(log_momentum)
    s = math.log1p(math.exp(float(log_scale))) + 0.01

    gm = np.ascontiguousarray(
        np.broadcast_to((gain * mix[:, None].astype(np.float32)).astype(np.float32)[:, None, :], (R, P, D)))
    bb = np.ascontiguousarray(np.broadcast_to(bias[:, None, :], (R, P, D)))
    scl = np.zeros((P, 4), np.float32)
    scl[:, 0] = mom
    scl[:, 1] = s * (1.0 - mom)
    scl[:, 2] = s * mom
    scl[:, 3] = 1.0 - mom
    idb = np.eye(128, dtype=BF16_NP)
    idh = np.eye(128, dtype=FP16_NP)
    p_ = np.arange(P)
    dmsk = np.where(np.arange(128)[None, :] <= p_[:, None], 0.0, NEG_MASK).astype(BF16_NP)

    def transposed_cols(rows_bf):  # [nb,128,D] bf16 -> [128, nb*D] chunk-T
        nb = rows_bf.shape[0]
        return np.ascontiguousarray(
            rows_bf.reshape(nb, 128, DC, 128).transpose(3, 0, 2, 1)).reshape(128, nb * D)

    in_maps = []
    for c in range(cfg.n_cores):
        bidx, cc = divmod(c, G)
        h0 = x[bidx]                       # [T, D]
        hblk = h0.reshape(NBLK, 128, D)
        hi_f = h0.astype(BF16_NP)
        lo_f = (h0 - hi_f.astype(np.float32)).astype(BF16_NP)
        hTh0 = np.ascontiguousarray(
            hi_f.T.reshape(DC, 128, T).transpose(1, 0, 2)).reshape(128, DC * T)
        hTl0 = np.ascontiguousarray(
            lo_f.T.reshape(DC, 128, T).transpose(1, 0, 2)).reshape(128, DC * T)
        hrows0 = np.ascontiguousarray(
            h0.astype(FP16_NP).reshape(NBLK, 128, D).transpose(1, 0, 2)).reshape(128, NBLK * D)
        tblocks = [cfg.tail_block(t, cc) for t in range(NT)]
        blocks = list(range(MR)) + tblocks  # myh slice -> block
        myh0 = np.ascontiguousarray(hblk[blocks].transpose(1, 0, 2)).reshape(128, NS * D)
        hTmh0 = transposed_cols(hi_f.reshape(NBLK, 128, D)[tblocks])
        hTml0 = transposed_cols(lo_f.reshape(NBLK, 128, D)[tblocks])
        # tail masks: slot t's masked region = its padded window's last
        # G*128 cols; allow iff absolute j <= blk*128 + p
        tmsk = np.zeros((P, NT * G * 128), BF16_NP)
        for t in range(NT):
            jj = np.arange(G * 128)
            tmsk[:, t * G * 128:(t + 1) * G * 128] = np.where(
                jj[None, :] <= cc * 128 + p_[:, None], 0.0, NEG_MASK).astype(BF16_NP)
        w2 = np.zeros((P, R * NS), np.float32)
        for r in range(R):
            for sl in range(NS):
                blk = blocks[sl]
                cnt = np.minimum(blk * 128 + p_ + 1, K_SCHEDULE[r])
                w2[:, r * NS + sl] = ((1.0 - mix[r]) / (mix[r] * cnt)).astype(np.float32)
        in_maps.append({
            "i_hTh": hTh0, "i_hTl": hTl0, "i_hrows": hrows0, "i_myh": myh0,
            "i_hTmh": hTmh0, "i_hTml": hTml0,
            "i_dmsk": dmsk, "i_tmsk": tmsk,
            "i_xs": (s * myh0.astype(np.float64)).astype(np.float32),
            "i_gm": gm, "i_bb": bb, "i_w2": w2, "i_sc": scl,
            "i_idb": idb, "i_idh": idh,
        })
    return in_maps


def assemble_output(cfg: Cfg, results, dtype=np.float32):
    """results: list (per core) of {'o_out': [NS,128,D]} -> full [B,T,D]."""
    out = np.zeros((cfg.B, cfg.T, cfg.D), dtype)
    for bidx in range(cfg.B):
        base = bidx * cfg.G
        o0 = results[base]["o_out"]
        for blk in range(cfg.MR):
            out[bidx, blk * 128:(blk + 1) * 128] = o0[blk]
        for cc in range(cfg.G):
            for t in range(cfg.NT):
                blk = cfg.tail_block(t, cc)
                out[bidx, blk * 128:(blk + 1) * 128] = results[base + cc]["o_out"][cfg.MR + t]
    return out


_PROGRAM_CACHE = {}


def _get_program(cfg: Cfg):
    key = (cfg.B, cfg.T, cfg.D, cfg.G, cfg.MR)
    if key not in _PROGRAM_CACHE:
        _PROGRAM_CACHE[key] = build_program(cfg)
    return _PROGRAM_CACHE[key]


def run(cfg: Cfg, inputs: dict, trace: bool = False):
    nc = _get_program(cfg)
    in_maps = prep_inputs(cfg, **inputs)
    res = bass_utils.run_bass_kernel_spmd(
        nc, in_maps, list(range(cfg.n_cores)), trace=trace)
    out = assemble_output(cfg, res.results)
    return out, res


def kernel(**inputs) -> np.ndarray:
    cfg = Cfg()  # B=2, T=2048, D=1024, 8 cores
    out, _ = run(cfg, inputs)
    return out.astype(np.float32)
